# revision 11
# baseline (speedup 1.0000x reference)
"""Trainium2 Bass kernel for nn_MinervaEnhancedLossV3.

Contract: kernel(**inputs) takes FULL unsharded inputs (B=2048), shards
batch-wise across 8 NeuronCores, runs one SPMD Bass program, and combines
per-batch partial statistics on the host into the scalar loss.

Device algorithm (per core, 264 padded batches = 22 groups of 12 = 11 pairs):
  layout: group tiles [120, 2304] with partition p = b_local*10 + c,
          free axis = H*W positions; pairs of groups side by side [120, 4608].
  host pre-converts pred to fp16 and gathers pv16 = fp16(pred[t]).
  e16 = exp(x16)                                 ACT
  sum_ps[q,pos] = 0.5 * sum_c e16 (packed rows)  PE fp16 (0.5-weight lhs)
  rep[p,pos] = pv[b(p),pos]                      PE bcast (PSUM) or DMA replicate (SBUF)
  geq16 = [x16 >= rep]                           DVE TT (2x vs SBUF, 1x vs PSUM)
  gcnt_ps = 0.5 * sum_c geq16                    PE fp16 (same 0.5-weight lhs)
  lse' = Ln(sum_ps); ce = max(lse'-pv+ln2, 0)    ACT + Pool sub + DVE ts
  pt = Exp(-ce); lu = Ln(1.0000001-pt)           ACT (bias folds)
  p25 = Exp(2.5*lu); fsum += p25*ce              ACT + DVE stt accum
  eq = [gcnt==0.5] (accum eqc); iou += eq*sw     DVE ts-accum + DVE stt accum
Host: focal weights w(unique,transitions), ultra_teal, exact bonus,
  copy-penalty (iterative candidate filtering + exact resolve), bonuses,
  nan/inf guard.
"""

import os
from contextlib import ExitStack

import numpy as np

import concourse.bass as bass
import concourse.bacc as bacc
import concourse.tile as tile
import concourse.mybir as mybir
from concourse.bass_utils import run_bass_kernel_spmd

F16 = mybir.dt.float16
F32 = mybir.dt.float32
AF = mybir.ActivationFunctionType
OP = mybir.AluOpType

N_CORES = 8
B_FULL = 2048
C = 10
H = W = 48
HW = H * W                      # 2304
BG = 12                         # batches per group
P = BG * C                      # 120 partitions per group tile
NPAIR = 11                      # group pairs per core (22 groups)
B_PC = 264                      # padded per-core batch
BPC = 256                       # real per-core batch
LN2 = float(np.float32(0.6931471805599453))

# pairs whose rep comes from DMA replication (geq at DVE 2x); rest use PE
# broadcast into PSUM (geq at DVE 1x)
GEQ_DMA_PAIRS = frozenset({0, 1, 2, 4, 5, 7, 8, 10})

SG_GROUPS = [2, 10, 10]                  # groups per supergroup (small first)
DHALVES = [(0, 1024), (1024, 1280)]      # PSUM half-tiles (start, len)
LAST_EXEC_NS = None


def _spatial_weights():
    cy, cx = H // 2, W // 2
    yy = np.arange(H, dtype=np.float64)[:, None]
    xx = np.arange(W, dtype=np.float64)[None, :]
    dist = np.sqrt((yy - cy) ** 2 + (xx - cx) ** 2)
    md = np.sqrt((H // 2) ** 2 + (W // 2) ** 2)
    return (1.0 + 0.3 * (1.0 - dist / md)).astype(np.float32)   # [H, W]


class ColMap:
    def __init__(self):
        self.n = 0
        self.m = {}

    def col(self, name):
        if name not in self.m:
            self.m[name] = self.n
            self.n += 1
        return self.m[name]


def build_nc(finalize=True):
    nc = bacc.Bacc(trn_type="TRN2") if finalize else bass.Bass(trn_type="TRN2")

    pred_in = nc.dram_tensor("pred16_in", [NPAIR, P, 2 * HW], F16, kind="ExternalInput")
    pv_in = nc.dram_tensor("pv_in", [B_PC, HW], F16, kind="ExternalInput")

    cm = ColMap()
    for sg in range(len(SG_GROUPS)):
        cm.col(f"fs_{sg}")
        cm.col(f"iou_{sg}")
        for k in range(2):
            cm.col(f"eqc_{sg}_{k}")
    ncols = max(cm.n, 16)
    out_cols = nc.dram_tensor("out_cols", [P, ncols], F32, kind="ExternalOutput")

    # ---- inline constants ----
    sw = np.repeat(_spatial_weights().reshape(1, HW), P, axis=0).astype(np.float16)
    sw_const = nc.inline_tensor(sw, name="sw_const")                     # [P, HW]

    # bca: [k=q rows(120), 10 groups * 120 partitions] fp16;
    # bca[q, gl*P + p] = 1 iff q == 12*gl + p//10  (broadcast pv row to channels)
    bca = np.zeros((P, 10 * P), dtype=np.float16)
    for gl in range(10):
        for b in range(BG):
            for c in range(C):
                bca[BG * gl + b, gl * P + b * C + c] = 1.0
    bca_const = nc.inline_tensor(bca, name="bca_const")

    # lhs16h: 0.5-valued channel-sum weights, [k=p(120), 10 groups * 120 rows]
    # lhs16h[p, gl*P + m] = 0.5 iff m == 12*gl + p//10
    lhs16h = np.zeros((P, 10 * P), dtype=np.float16)
    for gl in range(10):
        for b in range(BG):
            for c in range(C):
                lhs16h[b * C + c, gl * P + BG * gl + b] = 0.5
    lhs_const = nc.inline_tensor(lhs16h, name="lhs_const")

    with tile.TileContext(nc) as tc, ExitStack() as es:
        _emit(es, tc, nc, cm, pred_in, pv_in, out_cols,
              sw_const, bca_const, lhs_const)
    if finalize:
        nc.finalize()
    return nc, cm


def _emit(es, tc, nc, cm, pred_in, pv_in, out_cols,
          sw_const, bca_const, lhs_const):
    dma = nc.sync.dma_start

    singles = es.enter_context(tc.tile_pool(name="singles", bufs=1))
    xpool = es.enter_context(tc.tile_pool(name="xpool", bufs=5))
    epool = es.enter_context(tc.tile_pool(name="epool", bufs=3))
    gpool = es.enter_context(tc.tile_pool(name="gpool", bufs=6))
    pvpool = es.enter_context(tc.tile_pool(name="pvpool", bufs=2))
    pix = es.enter_context(tc.tile_pool(name="pix", bufs=1))
    scr = es.enter_context(tc.tile_pool(name="scr", bufs=1))
    reppool = es.enter_context(tc.tile_pool(name="reppool", bufs=2))
    ps_a = es.enter_context(tc.tile_pool(name="ps_a", bufs=1, space="PSUM"))
    ps_b = es.enter_context(tc.tile_pool(name="ps_b", bufs=1, space="PSUM"))
    ps_rep = es.enter_context(tc.tile_pool(name="ps_rep", bufs=1, space="PSUM"))

    # prefetch first data tiles before most constants so the DMA device
    # starts on pred immediately; lhs is needed by the first sum matmul
    x_first = xpool.tile([P, 2 * HW], F16, tag="x")
    dma(out=x_first[:], in_=pred_in[0, :, :])
    x_second = xpool.tile([P, 2 * HW], F16, tag="x")
    dma(out=x_second[:], in_=pred_in[1, :, :])
    pv_first = pvpool.tile([P, HW], F16, tag="pv")
    R0 = SG_GROUPS[0] * BG
    dma(out=pv_first[:R0], in_=pv_in[0:R0, :])
    lhs_t = singles.tile([P, 10 * P], F16)
    dma(out=lhs_t[:], in_=lhs_const[:, :])
    # sw/bca DMAs are emitted after the first pair's rep loads (see loop)
    sw_t = singles.tile([P, HW], F16)
    bca_t = singles.tile([P, 10 * P], F16)

    b_1eps = singles.tile([P, 1], F32, tag="b_1eps")
    nc.vector.memset(b_1eps[:], 1.0000001)

    colstage = singles.tile([P, max(cm.n, 16)], F32, tag="colstage")
    nc.vector.memset(colstage[:], 0.0)

    def ccol(name, r):
        return colstage[:r, cm.col(name):cm.col(name) + 1]

    pair0 = 0
    for sg, G in enumerate(SG_GROUPS):
        npair = G // 2
        R = G * BG
        sgb = pair0 * 2 * BG

        if sg == 0:
            pv_sg = pv_first
        else:
            pv_sg = pvpool.tile([P, HW], F16, tag="pv")
            dma(out=pv_sg[:R], in_=pv_in[sgb:sgb + R, :])

        sum_ha = ps_a.tile([P, DHALVES[0][1]], F32, tag="ps_a")
        sum_hb = ps_b.tile([P, DHALVES[1][1]], F32, tag="ps_b")
        sum_h = [sum_ha, sum_hb]
        geq_tiles = []
        for jj in range(npair):
            pj = pair0 + jj
            if pj == 0:
                x_t = x_first
            elif pj == 1:
                x_t = x_second
            else:
                x_t = xpool.tile([P, 2 * HW], F16, tag="x")
                dma(out=x_t[:], in_=pred_in[pj, :, :])

            # ---- exp ----
            e_t = epool.tile([P, 2 * HW], F16, tag="e")
            nc.scalar.activation(e_t[:], x_t[:], AF.Exp)

            # ---- sumexp matmuls (0.5 weights, accumulate over pairs) ----
            first = jj == 0
            last = jj == npair - 1
            for t in range(2):
                gl = 2 * jj + t
                lw = lhs_t[:, gl * P:(gl + 1) * P]
                for hi, (h0, hn) in enumerate(DHALVES):
                    for c0 in range(0, hn, 512):
                        cn = min(512, hn - c0)
                        nc.tensor.matmul(
                            sum_h[hi][:, c0:c0 + cn], lw,
                            e_t[:, t * HW + h0 + c0:t * HW + h0 + c0 + cn],
                            start=(first and t == 0), stop=(last and t == 1))

            # ---- rep broadcast + geq per group ----
            g_t = gpool.tile([P, 2 * HW], F16, tag="g")
            for t in range(2):
                gl = 2 * jj + t
                if pj in GEQ_DMA_PAIRS:
                    rep_sb = reppool.tile([P, HW], F16, tag="rep_sb")
                    base = pv_in[sgb + gl * BG:sgb + (gl + 1) * BG, :]
                    rep_src = bass.AP(tensor=base.tensor, offset=base.offset,
                                      ap=[base.ap[0], [0, C], base.ap[1]])
                    dma(out=rep_sb[:, :], in_=rep_src)
                    nc.vector.tensor_tensor(
                        out=g_t[:, t * HW:(t + 1) * HW],
                        in0=x_t[:, t * HW:(t + 1) * HW],
                        in1=rep_sb[:, :], op=OP.is_ge)
                else:
                    bcl = bca_t[0:R, gl * P:(gl + 1) * P]
                    for c0 in range(0, HW, 1024):
                        cn = min(1024, HW - c0)
                        rep_ps = ps_rep.tile([P, 1024], F32, tag="rep")
                        for k0 in range(0, cn, 512):
                            kn = min(512, cn - k0)
                            nc.tensor.matmul(
                                rep_ps[:, k0:k0 + kn], bcl,
                                pv_sg[0:R, c0 + k0:c0 + k0 + kn],
                                start=True, stop=True)
                        nc.vector.tensor_tensor(
                            out=g_t[:, t * HW + c0:t * HW + c0 + cn],
                            in0=x_t[:, t * HW + c0:t * HW + c0 + cn],
                            in1=rep_ps[:, :cn], op=OP.is_ge)
            geq_tiles.append(g_t)
            if pair0 + jj == 0:
                dma(out=sw_t[:], in_=sw_const[:, :])
                dma(out=bca_t[:], in_=bca_const[:, :])

        # ---- lse + focal chain (packed [R, HW]) ----
        lse = pix.tile([P, HW], F16, tag="lse")
        for hi, (h0, hn) in enumerate(DHALVES):
            nc.scalar.activation(lse[:R, h0:h0 + hn], sum_h[hi][0:R, :hn], AF.Ln)
        ce_raw = pix.tile([P, HW], F16, tag="ce_raw")
        nc.gpsimd.tensor_tensor(out=ce_raw[:R], in0=lse[:R], in1=pv_sg[:R],
                                op=OP.subtract)
        ce = pix.tile([P, HW], F16, tag="ce")
        nc.vector.tensor_scalar(out=ce[:R], in0=ce_raw[:R], scalar1=LN2,
                                scalar2=0.0, op0=OP.add, op1=OP.max)
        pt = pix.tile([P, HW], F16, tag="pt")
        nc.scalar.activation(pt[:R], ce[:R], AF.Exp, scale=-1.0)
        lu = pix.tile([P, HW], F16, tag="lu")
        nc.scalar.activation(lu[:R], pt[:R], AF.Ln, bias=b_1eps[:R], scale=-1.0)
        p25 = pix.tile([P, HW], F16, tag="p25")
        nc.scalar.activation(p25[:R], lu[:R], AF.Exp, scale=2.5)
        fs_scr = scr.tile([P, HW], F16, tag="fs_scr")
        nc.vector.scalar_tensor_tensor(
            out=fs_scr[:R], in0=p25[:R], scalar=0.0, in1=ce[:R],
            op0=OP.bypass, op1=OP.mult, accum_out=ccol(f"fs_{sg}", R))

        # ---- gcnt + eq (per PSUM half) + iou ----
        eq16 = scr.tile([P, HW], F16, tag="eq16")
        for hi, (h0, hn) in enumerate(DHALVES):
            pool_h = ps_a if hi == 0 else ps_b
            gcnt_h = pool_h.tile([P, hn], F32, tag="ps_a" if hi == 0 else "ps_b")
            for jj in range(npair):
                g_t = geq_tiles[jj]
                first = jj == 0
                last = jj == npair - 1
                for t in range(2):
                    gl = 2 * jj + t
                    lw = lhs_t[:, gl * P:(gl + 1) * P]
                    for c0 in range(0, hn, 512):
                        cn = min(512, hn - c0)
                        nc.tensor.matmul(
                            gcnt_h[:, c0:c0 + cn], lw,
                            g_t[:, t * HW + h0 + c0:t * HW + h0 + c0 + cn],
                            start=(first and t == 0), stop=(last and t == 1))
            nc.vector.tensor_scalar(
                out=eq16[:R, h0:h0 + hn], in0=gcnt_h[0:R, :hn],
                scalar1=0.5, scalar2=None, op0=OP.is_equal, op1=OP.add,
                accum_out=ccol(f"eqc_{sg}_{hi}", R))
        iou_scr = scr.tile([P, HW], F16, tag="iou_scr")
        nc.vector.scalar_tensor_tensor(
            out=iou_scr[:R], in0=eq16[:R], scalar=0.0, in1=sw_t[:R],
            op0=OP.bypass, op1=OP.mult, accum_out=ccol(f"iou_{sg}", R))

        pair0 += npair

    dma(out=out_cols[:, :], in_=colstage[:])


_NC_CACHE = {}


def _get_nc():
    if "nc" not in _NC_CACHE:
        _NC_CACHE["nc"] = build_nc(finalize=True)
    return _NC_CACHE["nc"]


def _host_stats(pred, targets, inputs_arr):
    """w weights, copy penalty; pure numpy."""
    B = pred.shape[0]
    t2 = targets.reshape(B, HW)
    pres = np.zeros((B, C), bool)
    pres[np.arange(B)[:, None], t2] = True
    uniq = pres.sum(1)
    trans = (targets[:, :, 1:] != targets[:, :, :-1]).sum((1, 2)) + \
            (targets[:, 1:, :] != targets[:, :-1, :]).sum((1, 2))
    w = np.where(uniq > 4, 1.3, 1.0) * np.where(trans > W, 1.2, 1.0)

    # copy penalty: iterative candidate filtering, then exact resolve
    pr2 = pred.reshape(B, C, HW)
    inp2 = inputs_arr.reshape(B, HW)
    cand = np.arange(B)
    for pos in range(64):
        if cand.size == 0:
            break
        am = pr2[cand, :, pos].argmax(1)
        cand = cand[am == inp2[cand, pos]]
    copy = np.zeros(B, np.float64)
    if cand.size:
        am = pr2[cand].argmax(1)
        copy[cand] = (am == inp2[cand]).all(1).astype(np.float64)
    return w, copy


def _combine(res_list, cm, w, copy, sf, ps, rd):
    B = B_FULL
    fsum = np.zeros(B, np.float64)
    iou_s = np.zeros(B, np.float64)
    eqc = np.zeros(B, np.float64)

    sg_bases = np.concatenate([[0], np.cumsum(np.array(SG_GROUPS) * BG)])
    for core, r in enumerate(res_list):
        cols = r["out_cols"]                        # [P, ncols]
        sl0 = core * BPC
        for sg in range(len(SG_GROUPS)):
            R = SG_GROUPS[sg] * BG
            sgb = int(sg_bases[sg])                 # per-core padded batch base
            rows = np.arange(R)
            gb = sgb + rows
            valid = gb < BPC
            bidx = sl0 + gb[valid]
            fsum[bidx] = cols[:R, cm.col(f"fs_{sg}")][valid]
            iou_s[bidx] = cols[:R, cm.col(f"iou_{sg}")][valid]
            e = sum(cols[:R, cm.col(f"eqc_{sg}_{k}")] for k in range(2))
            eqc[bidx] = e[valid]

    sw64 = _spatial_weights().astype(np.float64)
    SW = sw64.sum()
    focal = (fsum * w).sum() / (B * HW)

    strict = np.rint(eqc) == HW
    iou = iou_s / SW
    ut = 0.85 * iou + 0.15 * strict
    ut_mean = ut.mean()
    exact_bonus = max(-ut_mean * 5.0, -5.0)
    transform_penalty = copy.mean() * 0.5

    sf64 = sf.astype(np.float64)
    creativity = 1.0 / (1.0 + np.exp(-sf64.mean())) * 0.1
    strategic = ps.astype(np.float64).mean() * 0.1
    multi = rd.astype(np.float64).mean() * 0.1
    complexity = ut_mean * (HW / 1225.0) * 0.1

    total = (focal + transform_penalty + exact_bonus
             - creativity - strategic - multi - complexity)
    if np.isnan(total) or np.isinf(total):
        total = min(focal, 10.0)
    return np.float32(total)


def _prep_core_inputs(pred16, pv16):
    """pred16 [B, C, HW] fp16 -> per-core pair layout [NPAIR, P, 2*HW]."""
    in_maps = []
    for core in range(N_CORES):
        sl = slice(core * BPC, (core + 1) * BPC)
        pc = pred16[sl]                              # [256, C, HW]
        pvc = pv16[sl]                               # [256, HW]
        pad = B_PC - BPC
        pc = np.concatenate([pc, np.broadcast_to(pc[:1], (pad, C, HW))], 0)
        pvc = np.concatenate([pvc, np.broadcast_to(pvc[:1], (pad, HW))], 0)
        gt = pc.reshape(22, BG * C, HW)
        pairs = np.concatenate([gt[0::2], gt[1::2]], axis=2)   # [11, 120, 2*HW]
        in_maps.append({
            "pred16_in": np.ascontiguousarray(pairs),
            "pv_in": np.ascontiguousarray(pvc),
        })
    return in_maps


def _coresim_ns(in_map0):
    """CoreSim cost-model estimate of the single-core program."""
    import concourse.bass_interp as bass_interp
    nc, _cm = build_nc(finalize=False)
    sim = bass_interp.MultiCoreSim(nc, 1)
    core = sim.cores[0]
    core.publish_trace = False
    core.tensor("pred16_in")[:] = in_map0["pred16_in"]
    core.tensor("pv_in")[:] = in_map0["pv_in"]
    sim.simulate()
    return int(sim.global_time)


def kernel(pred, strategic_features, planning_score, reasoning_depth,
           targets, inputs):
    global LAST_EXEC_NS
    pred = np.ascontiguousarray(np.asarray(pred, dtype=np.float32))
    targets = np.ascontiguousarray(np.asarray(targets, dtype=np.int32))
    inputs_arr = np.ascontiguousarray(np.asarray(inputs, dtype=np.int32))
    sf = np.asarray(strategic_features, dtype=np.float32)
    ps = np.asarray(planning_score, dtype=np.float32)
    rd = np.asarray(reasoning_depth, dtype=np.float32)

    B = pred.shape[0]
    pr = pred.reshape(B, C, HW)
    t2 = targets.reshape(B, HW)

    pred16 = pr.astype(np.float16)
    pv16 = np.take_along_axis(pr, t2[:, None, :], axis=1)[:, 0].astype(np.float16)

    w, copy = _host_stats(pred, targets, inputs_arr)

    in_maps = _prep_core_inputs(pred16, pv16)

    nc, cm = _get_nc()
    trace = os.environ.get("BASSLOSS_TRACE", "0") == "1"
    res = run_bass_kernel_spmd(nc, in_maps, list(range(N_CORES)), trace=trace)
    LAST_EXEC_NS = res.exec_time_ns
    if LAST_EXEC_NS is None:
        try:
            LAST_EXEC_NS = _coresim_ns(in_maps[0])
        except Exception:
            LAST_EXEC_NS = None

    return _combine(res.results, cm, w, copy, sf, ps, rd)


if __name__ == "__main__":
    d = np.load("/root/problem/inputs_cache.npz")
    out = kernel(**{k: d[k] for k in d.files})
    print("kernel out:", out, " exec_ns:", LAST_EXEC_NS)
